# revision 1
# baseline (speedup 1.0000x reference)
"""Trainium2 Bass kernel for nn_DynamicMLP (3-layer LIF spiking net, T=16).

Strategy (8 NeuronCores, data-parallel over batch):
  - Shard batch 1024 -> 8 x 128. Replicate weights. Zero cross-core comms.
  - Layout: [batch=128 partitions, hidden on free dim].
  - The LIF current state c lives ENTIRELY in PSUM, scaled by 2^t:
      C_t = sum_{tau<=t} 2^tau * I_tau  ==  2^t * c_t  (bitwise-equivalent to the
      reference's c = 0.5*c + I decay, since powers of 2 are exact).
    Inputs are pre-scaled by 2^t on host (x) / on device (spikes).
  - The output is chaotically sensitive (1e-6 current noise -> 2% output
    error), so matmuls must be fp32-exact. They run as fp16 multi-term splits
    (fp16 x fp16 products are exact in fp32 PSUM accumulation; all stored
    operands kept in fp16 normal range; ~1e-7 residual):
      L0: x = xh + xl exactly (fp16 pair). 2^t*xh@wh -> C0;
          2^(t+11)*xl@wh and 2^t*xh@(wl*2^11) -> C0b (folded at 2^-(t+11)).
      L1/L2: spikes s*2^t are fp16-exact; s_hi@wh -> C and
          (s_hi*2^-11)@(wl*2^11) -> C, same scale, no extra banks.
    Residual error ~2e-8 per current, inside the fp32 matmul-order envelope.
  - Spikes are emitted as fp16 * 2^t and DMA-transposed (xbar) to become the
    next layer's stationary operand. Biases enter via a K=2 matmul row pair
    (rows scaled 2^t and 2^(t-11) for the hi/lo bias split).
"""
import sys

sys.path.insert(0, "/opt/trn_rl_repo")

import numpy as np

import concourse.bacc as bacc
import concourse.tile as tile
from concourse import mybir
from concourse.bass_utils import run_bass_kernel_spmd

dt = mybir.dt
F16 = dt.float16
F32 = dt.float32
Alu = mybir.AluOpType

NCORES = 8
FULL = dict(T=16, IN=2048, H0=1024, H1=1024, OUT=512, BL=128)
EXACT_ORDER = True  # reproduce the reference LIF rounding order exactly

_BUILD_CACHE = {}


def build(T=16, IN=2048, H0=1024, H1=1024, OUT=512, BL=128):
    key = (T, IN, H0, H1, OUT, BL, EXACT_ORDER)
    if key in _BUILD_CACHE:
        return _BUILD_CACHE[key]
    KT0, KT1, KT2 = IN // 128, H0 // 128, H1 // 128
    NCH = 512  # psum bank free-dim (fp32)

    nc = bacc.Bacc("TRN2", target_bir_lowering=False, debug=False, num_devices=NCORES)

    xa_d = nc.dram_tensor("xa", [T, IN, BL], F16, kind="ExternalInput")
    xr_d = nc.dram_tensor("xr", [T, IN, BL], F16, kind="ExternalInput")
    w_d = {}
    for nm, (a, b) in {"w0": (IN, H0), "w1": (H0, H1), "wo": (H1, OUT)}.items():
        w_d[nm + "a"] = nc.dram_tensor(nm + "a", [a, b], F16, kind="ExternalInput")
        w_d[nm + "l"] = nc.dram_tensor(nm + "l", [a, b], F16, kind="ExternalInput")
    b_d = {}
    for nm, h in {"b0": H0, "b1": H1, "b2": OUT}.items():
        b_d[nm] = nc.dram_tensor(nm, [2, h], F16, kind="ExternalInput")
    ones_d = nc.dram_tensor("onesrows", [2, T * 128], F16, kind="ExternalInput")
    id_d = nc.dram_tensor("ident", [128, 128], F16, kind="ExternalInput")
    out_d = nc.dram_tensor("out", [BL, OUT], F32, kind="ExternalOutput")

    with tile.TileContext(nc) as tc:
        with tc.tile_pool(name="w", bufs=1) as wp, \
             tc.tile_pool(name="state", bufs=1) as sp, \
             tc.tile_pool(name="xs", bufs=3) as xp, \
             tc.tile_pool(name="spk", bufs=2) as kp, \
             tc.tile_pool(name="psum", bufs=1, space="PSUM") as pp:

            # ---- resident weights (DMA order = first-use order) ----
            KH = max(KT0 // 2, 1)
            NX0 = KT0 // KH
            w_sb = {}
            for nm, (kt, h) in {"w1": (KT1, H1), "wo": (KT2, OUT)}.items():
                for sfx in ("a", "l"):
                    w_sb[nm + sfx] = wp.tile([128, kt * h], F16, tag=nm + sfx,
                                             name=nm + sfx)
            # w0 in per-chunk tiles so L0 can start after the first chunk lands
            for sfx in ("a", "l"):
                w_sb["w0" + sfx] = [
                    wp.tile([128, KH * H0], F16, tag=f"w0{sfx}{ci}", name=f"w0{sfx}{ci}")
                    for ci in range(NX0)]

            def dma_weights(nm, kt, h):
                for sfx in ("a", "l"):
                    tl = w_sb[nm + sfx]
                    for k in range(kt):
                        if nm == "w0":
                            nc.sync.dma_start(
                                out=tl[k // KH][:, (k % KH) * h:(k % KH + 1) * h],
                                in_=w_d[nm + sfx][k * 128:(k + 1) * 128, :])
                        else:
                            nc.sync.dma_start(out=tl[:, k * h:(k + 1) * h],
                                              in_=w_d[nm + sfx][k * 128:(k + 1) * 128, :])

            b_sb = {}
            for nm, h in {"b0": H0, "b1": H1, "b2": OUT}.items():
                tl = wp.tile([2, h], F16, tag=nm, name=nm)
                nc.sync.dma_start(out=tl[:], in_=b_d[nm][:])
                b_sb[nm] = tl


            # ---- states (single-buffered; DVE program order serializes) ----
            HS = {0: H0, 1: H1, 2: OUT}
            st = {}
            for l in (0, 1, 2):
                for nm in ("v", "u0", "v0", "q"):
                    st[(l, nm)] = sp.tile([128, HS[l]], F32, tag=f"{nm}{l}", name=f"{nm}{l}")
            c021 = sp.tile([128, max(H0, H1)], F32, tag="c021")
            scrA = sp.tile([128, max(H0, H1)], F32, tag="scrA")
            scrB12 = sp.tile([128, H1], F32, tag="scrB12", name="scrB12")
            scrB0b = sp.tile([128, H0], F32, tag="scrB0b", name="scrB0b")
            scrB = {0: sp.tile([128, H0], F32, tag="scrB0", name="scrB0"),
                    1: scrB12, 2: scrB12}
            # psum current accumulators (2^t-scaled)
            C = {0: pp.tile([128, H0], F32, tag="C0", name="C0"),
                 1: pp.tile([128, H1], F32, tag="C1", name="C1"),
                 2: pp.tile([128, OUT], F32, tag="C2", name="C2")}
            C0b = pp.tile([128, H0], F32, tag="C0b", name="C0b")
            accP = pp.tile([128, OUT], F32, tag="accP", name="accP")
            ident = wp.tile([128, 128], F16, tag="ident", name="ident")
            nc.sync.dma_start(out=ident[:], in_=id_d[:])


            # ---- init ----
            for l in (0, 1, 2):
                for nm in ("v", "u0", "v0", "q"):
                    nc.vector.memset(st[(l, nm)][:], 0.0)
            nc.vector.memset(c021[:], 0.021)

            def lif_B(l, t):
                """Release C[l] (+C0b) into scratch on ACT (short queue, and
                the 2^-t scales are exact powers of two -> no rounding)."""
                h = HS[l]
                nc.scalar.mul(scrB[l][:, :h], C[l][:], float(2.0 ** -t))
                if l == 0:
                    nc.scalar.mul(scrB0b[:], C0b[:], float(2.0 ** -(t + 11)))

            def lif_ops(l, t, s_out, last=False):
                """Emit LIF elementwise ops for layer l at step t.

                Consumes C[l] (psum, = 2^t * c_t), states v0/u0/q from step t-1.
                Produces v (=v_t), updates u0/v0/q for t+1, and (if s_out) the
                2^t-scaled fp16 spike tensor.
                """
                h = HS[l]
                v, u0, v0, q = (st[(l, n)] for n in ("v", "u0", "v0", "q"))
                A = scrA[:, :h]
                if EXACT_ORDER:
                    B = scrB[l][:, :h]
                    if not last:
                        # u_t = u0 + ((-0.172*v0) + 0.529*u0)  (reference rounding)
                        nc.scalar.mul(A, v0[:], -0.172)
                        nc.vector.scalar_tensor_tensor(
                            out=A, in0=u0[:], scalar=0.529, in1=A,
                            op0=Alu.mult, op1=Alu.add)
                        nc.vector.tensor_tensor(out=A, in0=u0[:], in1=A, op=Alu.add)
                    # dv = ((q - v0) - u0) + c;  v = v0 + dv   (reference rounding)
                    nc.vector.tensor_tensor(out=v[:], in0=q[:], in1=v0[:],
                                            op=Alu.subtract)
                    nc.vector.tensor_tensor(out=v[:], in0=v[:], in1=u0[:],
                                            op=Alu.subtract)
                    if l == 0:
                        nc.vector.tensor_tensor(out=v[:], in0=v[:], in1=scrB0b[:],
                                                op=Alu.add)
                    nc.vector.tensor_tensor(out=v[:], in0=v[:], in1=B, op=Alu.add)
                    nc.vector.tensor_tensor(out=v[:], in0=v0[:], in1=v[:],
                                            op=Alu.add)
                else:
                    # u_t = 1.529*(u0 - (0.172/1.529)*v0)   (A := u_t)
                    nc.vector.scalar_tensor_tensor(
                        out=A, in0=v0[:], scalar=float(-0.172 / 1.529), in1=u0[:],
                        op0=Alu.mult, op1=Alu.add)
                    nc.vector.tensor_scalar(out=A, in0=A, scalar1=1.529,
                                            scalar2=None, op0=Alu.mult)
                    # v_t = (q - u0) + [2^-(t+11) * C0b] + 2^-t * C
                    nc.vector.tensor_tensor(out=v[:], in0=q[:], in1=u0[:],
                                            op=Alu.subtract)
                    if l == 0:
                        nc.vector.scalar_tensor_tensor(
                            out=v[:], in0=C0b[:], scalar=float(2.0 ** -(t + 11)),
                            in1=v[:], op0=Alu.mult, op1=Alu.add)
                    nc.vector.scalar_tensor_tensor(
                        out=v[:], in0=C[l][:], scalar=float(2.0 ** -t), in1=v[:],
                        op0=Alu.mult, op1=Alu.add)
                # spikes (scale 2^t for l<2; unscaled for l==2) -> fp16
                s_scale = 1.0 if l == 2 else float(2.0 ** t)
                nc.vector.tensor_scalar(out=s_out, in0=v[:], scalar1=0.5,
                                        scalar2=s_scale, op0=Alu.is_gt,
                                        op1=Alu.mult)
                if l == 2:
                    pending_acc.append((t, s_out))
                if last:
                    return
                # u0_{t+1} = u_t + 0.132 * s_t     (unscale s_out)
                nc.vector.scalar_tensor_tensor(
                    out=u0[:], in0=s_out, scalar=float(0.132 / s_scale), in1=A,
                    op0=Alu.mult, op1=Alu.add)
                # v0_{t+1} = v_t with 0.021 where spiked
                nc.scalar.copy(v0[:], v[:])
                nc.vector.copy_predicated(out=v0[:], mask=s_out.bitcast(dt.uint16),
                                          data=c021[:, :h])
                # q_{t+1} = v0^2
                nc.scalar.square(q[:], v0[:])

            def matmuls(l, t, kt, h, lhsA, lhsR, wa, wl, bias, ones2,
                        k_base=0, bias_too=True, kt_total=None):
                """Accumulate 2^t * (x@W + b) into C[l] (+C0b lo-part for l=0).

                l==0: lhsA = 2^t*xh tiles, lhsR = 2^(t+11)*xl tiles.
                      lhsA@wa -> C0; lhsR@wa -> C0b; lhsA@wl(*2^11) -> C0b.
                l>0:  lhsA = 2^t*s_hi tiles, lhsR = 2^(t-11)*s_hi tiles.
                      lhsA@wa -> C; lhsR@wl(*2^11) -> C.
                start=True is emitted per PSUM bank (each n0 chunk) at t==0.
                """
                kt_total = kt_total if kt_total is not None else kt
                for k in range(kt):
                    kg = k_base + k
                    for n0 in range(0, h, NCH):
                        nn = min(NCH, h - n0)
                        first = (t == 0 and kg == 0)
                        last = (t == T - 1 and kg == kt_total - 1)
                        ps = C[l][:, n0:n0 + nn]
                        ra = wa[:, k * h + n0: k * h + n0 + nn]
                        rl = wl[:, k * h + n0: k * h + n0 + nn]
                        la = lhsA[:, k * 128:(k + 1) * 128]
                        lr = lhsR[:, k * 128:(k + 1) * 128]
                        nc.tensor.matmul(ps, la, ra, start=first,
                                         stop=False, skip_group_check=True)
                        if l == 0:
                            psb = C0b[:, n0:n0 + nn]
                            nc.tensor.matmul(psb, lr, ra, start=first,
                                             stop=False, skip_group_check=True)
                            nc.tensor.matmul(psb, la, rl, start=False, stop=last,
                                             skip_group_check=True)
                        else:
                            nc.tensor.matmul(ps, lr, rl, start=False, stop=False,
                                             skip_group_check=True)
                if bias_too:
                    for n0 in range(0, h, NCH):
                        nn = min(NCH, h - n0)
                        nc.tensor.matmul(C[l][:, n0:n0 + nn], ones2[:],
                                         bias[:, n0:n0 + nn], start=False,
                                         stop=(t == T - 1), skip_group_check=True)

            ones2_h = {}
            pending_acc = []

            def flush_acc():
                while pending_acc:
                    ta, s2ap = pending_acc.pop(0)
                    nc.tensor.matmul(accP[:], ident[:], s2ap, start=(ta == 0),
                                     stop=(ta == T - 1), skip_group_check=True)

            x_pre = {}

            def load_x(t):
                ones2 = xp.tile([2, 128], F16, tag="ones2", name=f"ones2_t{t}")
                nc.sync.dma_start(out=ones2[:], in_=ones_d[:, t * 128:(t + 1) * 128])
                ones2_h[t] = ones2
                tiles = []
                for ci in range(NX0):
                    xa_t = xp.tile([128, KH * BL], F16, tag="xa", name=f"xa_t{t}_{ci}")
                    xr_t = xp.tile([128, KH * BL], F16, tag="xr", name=f"xr_t{t}_{ci}")
                    ks = ci * KH * 128
                    nc.sync.dma_start(
                        out=xa_t[:].rearrange("p (k b) -> p k b", b=BL),
                        in_=xa_d[t:t + 1, ks:ks + KH * 128].rearrange(
                            "o (k p) b -> p (o k) b", p=128))
                    nc.sync.dma_start(
                        out=xr_t[:].rearrange("p (k b) -> p k b", b=BL),
                        in_=xr_d[t:t + 1, ks:ks + KH * 128].rearrange(
                            "o (k p) b -> p (o k) b", p=128))
                    tiles.append((xa_t, xr_t))
                x_pre[t] = tiles

            def emit_L0(t, cis=None):
                if t not in x_pre:
                    load_x(t)
                tiles = x_pre[t]
                if cis is None or 1 in cis:
                    x_pre.pop(t, None)
                ones2 = ones2_h[t]
                for ci in (cis if cis is not None else range(NX0)):
                    xa_t, xr_t = tiles[ci]
                    matmuls(0, t, KH, H0, xa_t[:], xr_t[:],
                            w_sb["w0a"][ci][:], w_sb["w0l"][ci][:],
                            b_sb["b0"], ones2[:], k_base=ci * KH,
                            bias_too=(ci == NX0 - 1), kt_total=KT0)

            def emit_rest(t, filler=None):
                flush_acc()
                ones2 = ones2_h[t]
                s0 = kp.tile([128, H0], F16, tag="sPre", name=f"s0_t{t}")
                lif_ops(0, t, s0[:], last=(t == T - 1))  # B0 emitted by caller
                s0T = kp.tile([128, H0], F16, tag="sT", name=f"s0T_t{t}")
                nc.sync.dma_start_transpose(
                    out=s0T[:].rearrange("p (k b) -> p k b", b=128), in_=s0[:])
                s0L = kp.tile([128, H0], F16, tag="sL", name=f"s0L_t{t}", bufs=2)
                nc.vector.tensor_scalar(out=s0L[:], in0=s0T[:],
                                        scalar1=float(2.0 ** -11), scalar2=None,
                                        op0=Alu.mult)
                matmuls(1, t, KT1, H1, s0T[:], s0L[:], w_sb["w1a"], w_sb["w1l"],
                        b_sb["b1"], ones2[:])
                lif_B(1, t)
                if filler is not None:
                    filler()
                s1 = kp.tile([128, H1], F16, tag="sPre", name=f"s1_t{t}")
                lif_ops(1, t, s1[:], last=(t == T - 1))
                s1T = kp.tile([128, H1], F16, tag="sT", name=f"s1T_t{t}")
                nc.sync.dma_start_transpose(
                    out=s1T[:].rearrange("p (k b) -> p k b", b=128), in_=s1[:])
                s1L = kp.tile([128, H1], F16, tag="sL", name=f"s1L_t{t}", bufs=2)
                nc.vector.tensor_scalar(out=s1L[:], in0=s1T[:],
                                        scalar1=float(2.0 ** -11), scalar2=None,
                                        op0=Alu.mult)
                matmuls(2, t, KT2, OUT, s1T[:], s1L[:], w_sb["woa"], w_sb["wol"],
                        b_sb["b2"], ones2[:])
                lif_B(2, t)
                s2 = kp.tile([128, OUT], F16, tag="s2", name=f"s2_t{t}", bufs=2)
                lif_ops(2, t, s2[:], last=(t == T - 1))
                ones2_h.pop(t, None)

            # preamble DMAs in first-use order: x(0) first, then weights
            load_x(0)
            for ci in range(NX0):
                for sfx in ("a", "l"):
                    tl = w_sb["w0" + sfx][ci]
                    for kk in range(KH):
                        k = ci * KH + kk
                        nc.sync.dma_start(out=tl[:, kk * H0:(kk + 1) * H0],
                                          in_=w_d["w0" + sfx][k * 128:(k + 1) * 128, :])
            dma_weights("w1", KT1, H1)
            dma_weights("wo", KT2, OUT)

            # 1-step layer skew: PE gets L0(t+1) while the t chain drains
            for t in range(T):
                if t >= 1:
                    lif_B(0, t - 1)       # free C0/C0b for step t's matmuls
                emit_L0(t, cis=(0,))
                if t + 1 < T:
                    load_x(t + 1)
                if t >= 1:
                    emit_rest(t - 1, filler=lambda tt=t: emit_L0(tt, cis=(1,)))
                else:
                    emit_L0(t, cis=(1,))
            lif_B(0, T - 1)
            emit_rest(T - 1)

            flush_acc()
            accS = sp.tile([128, OUT], F32, tag="accS", name="accS")
            nc.vector.tensor_copy(out=accS[:], in_=accP[:])
            nc.sync.dma_start(out=out_d[:], in_=accS[:])

    nc.compile()
    _BUILD_CACHE[key] = nc
    return nc


def _split_f16(a32, lo_scale=2048.0):
    """a32 ~ hi + lo*2^-11 with hi = fp16(a32), lo = fp16((a32-hi)*2^11)."""
    hi = a32.astype(np.float16)
    lo = ((a32 - hi.astype(np.float32)) * np.float32(lo_scale)).astype(np.float16)
    return hi, lo


def prep_inputs(in_pop_spikes, W0, b0, W1, b1, Wout, bout,
                T=16, BL=128, ncores=NCORES):
    """Host-side prep: transpose/scale/split x, split weights; 8 in_maps."""
    x = np.ascontiguousarray(np.transpose(np.asarray(in_pop_spikes, np.float32),
                                          (2, 1, 0)))  # [T, IN, B]
    scale = (2.0 ** np.arange(T, dtype=np.float32)).reshape(T, 1, 1)
    xh32 = x.astype(np.float16).astype(np.float32)
    xa = (xh32 * scale).astype(np.float16)                 # exact 2^t * fp16(x)
    xr = ((x - xh32) * (scale * np.float32(2048.0))).astype(np.float16)
    # ^ 2^(t+11) * xl, fp16 (xl itself is the exact fp32 residual)

    com = {}
    for nm, W in (("w0", W0), ("w1", W1), ("wo", Wout)):
        WT = np.ascontiguousarray(np.asarray(W, np.float32).T)
        com[nm + "a"], com[nm + "l"] = _split_f16(WT)
    for nm, b in (("b0", b0), ("b1", b1), ("b2", bout)):
        hi, lo = _split_f16(np.asarray(b, np.float32))
        com[nm] = np.stack([hi, lo])

    T_ = T
    onesrows = np.zeros((2, T_ * 128), np.float16)
    for t in range(T_):
        onesrows[0, t * 128:(t + 1) * 128] = np.float16(2.0 ** t)
        onesrows[1, t * 128:(t + 1) * 128] = np.float16(2.0 ** (t - 11))
    com["onesrows"] = onesrows
    com["ident"] = np.eye(128, dtype=np.float16)

    in_maps = []
    for c in range(ncores):
        m = dict(com)
        m["xa"] = np.ascontiguousarray(xa[:, :, c * BL:(c + 1) * BL])
        m["xr"] = np.ascontiguousarray(xr[:, :, c * BL:(c + 1) * BL])
        in_maps.append(m)
    return in_maps


def kernel(in_pop_spikes, W0, b0, W1, b1, Wout, bout, batch_size, _trace=False):
    T = in_pop_spikes.shape[2]
    nc = build(**FULL)
    in_maps = prep_inputs(in_pop_spikes, W0, b0, W1, b1, Wout, bout, T=T)
    res = run_bass_kernel_spmd(nc, in_maps, core_ids=list(range(NCORES)),
                               trace=_trace)
    out = np.concatenate([r["out"] for r in res.results], axis=0)
    out = (out / np.float32(T)).astype(np.float32)
    if _trace:
        kernel._last_results = res
    return out



# revision 2
# speedup vs baseline: 1.0081x; 1.0081x over previous
"""Trainium2 Bass kernel for nn_DynamicMLP (3-layer LIF spiking net, T=16).

Strategy (8 NeuronCores, data-parallel over batch):
  - Shard batch 1024 -> 8 x 128. Replicate weights. Zero cross-core comms.
  - Layout: [batch=128 partitions, hidden on free dim].
  - The LIF current c lives in PSUM scaled by 2^t:
      C_t = sum_{tau<=t} 2^tau * I_tau  ==  2^t * (c_t - bias part)
    Bias is factored out of the recursion (fixed point):
      c_t = 2^-t * C_t + (2 - 2^-t) * b
  - fp32-exact matmuls via fp16 hi/lo splits (fp16 x fp16 exact in fp32 PSUM):
      L0 (x inexact in fp16): 3 passes  xh@wh -> C0; xl@wh, xh@(wl*2^11) -> C0b
      L1/L2 (spikes exact):   2 passes  s@wh -> C;  (s*2^-11)@(wl*2^11) -> C
  - LIF algebra refactored to minimize the post-matmul critical chain:
      v_t = v0^2 - 0.172*U0 + c_t         (U == u / 0.172)
      U_t = 1.529*U0 - v0 ; U_{t+1} = U_t + (0.132/0.172)*s_t
    r = v0^2 - 0.172*U0 and r2 = r + (2-2^-t)*b precompute BEFORE C is ready,
    so the chain after the last matmul is just stt(C) -> spike -> transpose.
  - Elementwise work is spread over DVE / ACT / Pool engines; the PE runs only
    the 144 GEMM instructions per step (the precision-mandated minimum).
  - Host packs x and W hi/lo interleaved so every DMA row is a >=512B run.
"""
import sys

sys.path.insert(0, "/opt/trn_rl_repo")

import numpy as np

import concourse.bacc as bacc
import concourse.tile as tile
from concourse import mybir
from concourse.bass_utils import run_bass_kernel_spmd

dt = mybir.dt
F16 = dt.float16
F32 = dt.float32
Alu = mybir.AluOpType

NCORES = 8
FULL = dict(T=16, IN=2048, H0=1024, H1=1024, OUT=512, BL=128)
TH_V = -0.172
TH_U = 0.529
TH_S = 0.132
C_RESET = 0.021

_BUILD_CACHE = {}


def build(T=16, IN=2048, H0=1024, H1=1024, OUT=512, BL=128):
    key = (T, IN, H0, H1, OUT, BL)
    if key in _BUILD_CACHE:
        return _BUILD_CACHE[key]
    KT0, KT1, KT2 = IN // 128, H0 // 128, H1 // 128
    KH = KT0 // 2          # ktiles per x chunk
    NCH = 512              # psum bank free-dim (fp32)
    HS = {0: H0, 1: H1, 2: OUT}
    KTS = {1: KT1, 2: KT2}
    HTOT = H0 + H1 + OUT
    BOFF = {0: 0, 1: H0, 2: H0 + H1}

    nc = bacc.Bacc("TRN2", target_bir_lowering=False, debug=False, num_devices=NCORES)

    xp_d = nc.dram_tensor("xp", [T, IN, 2 * BL], F16, kind="ExternalInput")
    w_d = {"w0": nc.dram_tensor("w0p", [IN, 2 * H0], F16, kind="ExternalInput"),
           "w1": nc.dram_tensor("w1p", [H0, 2 * H1], F16, kind="ExternalInput"),
           "wo": nc.dram_tensor("wop", [H1, 2 * OUT], F16, kind="ExternalInput")}
    b_d = nc.dram_tensor("brep2", [128, HTOT], F32, kind="ExternalInput")
    nb_d = nc.dram_tensor("negb", [2, HTOT], F16, kind="ExternalInput")
    o2_d = nc.dram_tensor("ones2", [2, 128], F16, kind="ExternalInput")
    out_d = nc.dram_tensor("out", [BL, OUT], F32, kind="ExternalOutput")

    with tile.TileContext(nc) as tc:
        with tc.tile_pool(name="w", bufs=1) as wp, \
             tc.tile_pool(name="state", bufs=1) as sp, \
             tc.tile_pool(name="xs", bufs=2) as xp, \
             tc.tile_pool(name="spk", bufs=2) as kp, \
             tc.tile_pool(name="psum", bufs=1, space="PSUM") as pp:

            # ---- resident weight tiles (DMAs issued later, first-use order) --
            w_sb = {"w0": [wp.tile([128, 2 * H0], F16, tag=f"w0_{k}", name=f"w0_{k}")
                           for k in range(KT0)],
                    "w1": [wp.tile([128, 2 * H1], F16, tag=f"w1_{k}", name=f"w1_{k}")
                           for k in range(KT1)],
                    "wo": [wp.tile([128, 2 * OUT], F16, tag=f"wo_{k}", name=f"wo_{k}")
                           for k in range(KT2)]}

            def dma_w(nm, ks):
                for k in ks:
                    nc.sync.dma_start(out=w_sb[nm][k][:],
                                      in_=w_d[nm][k * 128:(k + 1) * 128, :])

            bfull = wp.tile([128, HTOT], F32, tag="bfull", name="bfull")  # 2*b
            negb = wp.tile([2, HTOT], F16, tag="negb", name="negb")
            ones2 = wp.tile([2, 128], F16, tag="ones2", name="ones2")

            # ---- states ----
            vm = {l: sp.tile([128, HS[l]], F32, tag=f"vm{l}", name=f"vm{l}")
                  for l in range(3)}
            vr = {l: sp.tile([128, HS[l]], F32, tag=f"vr{l}", name=f"vr{l}")
                  for l in range(3)}
            U = {l: sp.tile([128, HS[l]], F32, tag=f"U{l}", name=f"U{l}")
                 for l in range(3)}
            r2 = {l: sp.tile([128, HS[l]], F32, tag=f"r2{l}", name=f"r2{l}")
                  for l in range(3)}
            r = sp.tile([128, max(H0, H1)], F32, tag="r", name="r")
            c021 = sp.tile([128, max(H0, H1)], F32, tag="c021", name="c021")
            acc = sp.tile([128, OUT], F32, tag="acc", name="acc")

            C = {0: pp.tile([128, H0], F32, tag="C0", name="C0"),
                 1: pp.tile([128, H1], F32, tag="C1", name="C1"),
                 2: pp.tile([128, OUT], F32, tag="C2", name="C2")}
            C0b = pp.tile([128, H0], F32, tag="C0b", name="C0b")

            # ---- x tile loads (2 chunks per step; 512B dram runs) ----
            x_pre = {}

            def load_x(t):
                tiles = []
                for ci in range(2):
                    xt = xp.tile([128, KH * 2 * BL], F16, tag=f"x{ci}",
                                 name=f"x_t{t}_{ci}")
                    ks = ci * KH * 128
                    nc.sync.dma_start(
                        out=xt[:].rearrange("p (k b) -> p k b", b=2 * BL),
                        in_=xp_d[t:t + 1, ks:ks + KH * 128, :].rearrange(
                            "o (k p) b -> p (o k) b", p=128))
                    tiles.append(xt)
                x_pre[t] = tiles

            # ---- matmul emitters ----
            def emit_L0(t, ci):
                xt = x_pre[t][ci]
                if ci == 1:
                    x_pre.pop(t, None)
                for k in range(KH):
                    kg = ci * KH + k
                    la = xt[:, k * 256:k * 256 + 128]
                    lr = xt[:, k * 256 + 128:(k + 1) * 256]
                    wt = w_sb["w0"][kg]
                    for n0 in range(0, H0, NCH):
                        first = (t == 0 and kg == 0)
                        last = (t == T - 1 and kg == KT0 - 1)
                        ra = wt[:, n0:n0 + NCH]
                        rl = wt[:, H0 + n0:H0 + n0 + NCH]
                        nc.tensor.matmul(C[0][:, n0:n0 + NCH], la, ra, start=False,
                                         stop=last, skip_group_check=True)
                        psb = C0b[:, n0:n0 + NCH]
                        nc.tensor.matmul(psb, lr, ra, start=first, stop=False,
                                         skip_group_check=True)
                        nc.tensor.matmul(psb, la, rl, start=False, stop=last,
                                         skip_group_check=True)

            def emit_L(l, t, sT, sL):
                h = HS[l]
                for k in range(KTS[l]):
                    la = sT[:, k * 128:(k + 1) * 128]
                    lr = sL[:, k * 128:(k + 1) * 128]
                    wt = w_sb["w1" if l == 1 else "wo"][k]
                    for n0 in range(0, h, NCH):
                        nn = min(NCH, h - n0)
                        last = (t == T - 1 and k == KTS[l] - 1)
                        ps = C[l][:, n0:n0 + nn]
                        nc.tensor.matmul(ps, la, wt[:, n0:n0 + nn], start=False,
                                         stop=False, skip_group_check=True)
                        nc.tensor.matmul(ps, lr, wt[:, h + n0:h + n0 + nn],
                                         start=False, stop=last,
                                         skip_group_check=True)

            def emit_bias_init():
                """Seed C0/C1/C2 with -b via a K=2 matmul (rows 1, 2^-11).
                Then c_t = 2^-t*C_t + 2b for all t."""
                for l in range(3):
                    h = HS[l]
                    for n0 in range(0, h, NCH):
                        nn = min(NCH, h - n0)
                        nc.tensor.matmul(
                            C[l][:, n0:n0 + nn], ones2[:],
                            negb[:, BOFF[l] + n0:BOFF[l] + n0 + nn],
                            start=True, stop=False, skip_group_check=True)

            # ---- LIF pieces ----
            def emit_E(l, t):
                """Precompute r2_l = vr^2 - 0.172*U + 2b and U_t = 1.529*U - vr.
                Runs before C[l] is ready."""
                h = HS[l]
                nc.scalar.square(r[:, :h], vr[l][:])
                nc.vector.scalar_tensor_tensor(
                    out=r[:, :h], in0=U[l][:], scalar=TH_V, in1=r[:, :h],
                    op0=Alu.mult, op1=Alu.add)
                nc.gpsimd.tensor_tensor(
                    out=r2[l][:], in0=bfull[:, BOFF[l]:BOFF[l] + h],
                    in1=r[:, :h], op=Alu.add)
                nc.gpsimd.tensor_scalar(out=U[l][:], in0=U[l][:],
                                        scalar1=float(1.0 + TH_U), scalar2=None,
                                        op0=Alu.mult)
                nc.gpsimd.tensor_tensor(out=U[l][:], in0=U[l][:], in1=vr[l][:],
                                        op=Alu.subtract)

            def vhead(l, t):
                """v_t = 2^-t*C (+2^-(t+11)*C0b) + r2 — frees C[l] for t+1."""
                if l == 0:
                    nc.vector.scalar_tensor_tensor(
                        out=vm[0][:], in0=C0b[:], scalar=float(2.0 ** -(t + 11)),
                        in1=r2[0][:], op0=Alu.mult, op1=Alu.add)
                    nc.vector.scalar_tensor_tensor(
                        out=vm[0][:], in0=C[0][:], scalar=float(2.0 ** -t),
                        in1=vm[0][:], op0=Alu.mult, op1=Alu.add)
                else:
                    nc.vector.scalar_tensor_tensor(
                        out=vm[l][:], in0=C[l][:], scalar=float(2.0 ** -t),
                        in1=r2[l][:], op0=Alu.mult, op1=Alu.add)

            def vrest(l, t):
                """spike + state updates + E(l, t+1); returns (sT, sL) for l<2."""
                h = HS[l]
                last = (t == T - 1)
                if l == 2:
                    s2 = kp.tile([128, OUT], F32, tag="s2", name=f"s2_t{t}",
                                 bufs=1)
                    nc.vector.tensor_scalar(out=s2[:], in0=vm[2][:], scalar1=0.5,
                                            scalar2=1.0, op0=Alu.is_gt,
                                            op1=Alu.mult)
                    nc.gpsimd.tensor_tensor(out=acc[:], in0=acc[:], in1=s2[:],
                                            op=Alu.add)
                    if not last:
                        nc.vector.scalar_tensor_tensor(
                            out=U[2][:], in0=s2[:], scalar=float(TH_S / -TH_V),
                            in1=U[2][:], op0=Alu.mult, op1=Alu.add)
                        nc.scalar.copy(vr[2][:], vm[2][:])
                        nc.vector.copy_predicated(
                            out=vr[2][:], mask=s2[:].bitcast(dt.uint32),
                            data=c021[:, :h])
                        emit_E(2, t + 1)
                    return None, None
                sc = float(2.0 ** t)
                s = kp.tile([128, h], F16, tag="s", name=f"s{l}_t{t}")
                nc.vector.tensor_scalar(out=s[:], in0=vm[l][:], scalar1=0.5,
                                        scalar2=sc, op0=Alu.is_gt, op1=Alu.mult)
                sT = kp.tile([128, h], F16, tag="sT", name=f"sT{l}_t{t}")
                nc.scalar.dma_start_transpose(
                    out=sT[:].rearrange("p (k b) -> p k b", b=128), in_=s[:])
                sL = kp.tile([128, h], F16, tag="s", name=f"sL{l}_t{t}")
                nc.scalar.mul(sL[:], sT[:], float(2.0 ** -11))
                if not last:
                    nc.vector.scalar_tensor_tensor(
                        out=U[l][:], in0=s[:], scalar=float(TH_S / -TH_V / sc),
                        in1=U[l][:], op0=Alu.mult, op1=Alu.add)
                    nc.scalar.copy(vr[l][:], vm[l][:])
                    nc.vector.copy_predicated(
                        out=vr[l][:], mask=s[:].bitcast(dt.uint16),
                        data=c021[:, :h])
                    emit_E(l, t + 1)
                return sT, sL

            # ---- preamble: DMAs in first-use order + state init ----
            nc.sync.dma_start(out=ones2[:], in_=o2_d[:])
            nc.sync.dma_start(out=negb[:], in_=nb_d[:])
            emit_bias_init()
            load_x(0)
            dma_w("w0", range(0, KH))
            nc.vector.memset(c021[:], C_RESET)
            nc.gpsimd.memset(acc[:], 0.0)
            for l in range(3):
                nc.vector.memset(vr[l][:], 0.0)
                nc.gpsimd.memset(U[l][:], 0.0)
            dma_w("w0", range(KH, KT0))
            nc.sync.dma_start(out=bfull[:], in_=b_d[:])
            load_x(1)
            dma_w("w1", range(KT1))
            dma_w("wo", range(KT2))
            for l in range(3):
                emit_E(l, 0)

            # ---- steady state: 1-step layer skew ----
            def emit_rest(t):
                sT0, sL0 = vrest(0, t)
                emit_L(1, t, sT0, sL0)
                if t + 1 < T:
                    emit_L0(t + 1, 1)
                vhead(1, t)
                sT1, sL1 = vrest(1, t)
                emit_L(2, t, sT1, sL1)
                vhead(2, t)
                vrest(2, t)

            for t in range(T):
                if t >= 1:
                    vhead(0, t - 1)
                emit_L0(t, 0)
                if t >= 1:
                    emit_rest(t - 1)
                else:
                    emit_L0(0, 1)
                if t + 2 < T:
                    load_x(t + 2)
            vhead(0, T - 1)
            emit_rest(T - 1)

            nc.sync.dma_start(out=out_d[:], in_=acc[:])

    nc.compile()
    _BUILD_CACHE[key] = nc
    return nc


def prep_inputs(in_pop_spikes, W0, b0, W1, b1, Wout, bout,
                T=16, BL=128, ncores=NCORES):
    """Host-side prep: pack hi/lo-split x and W with interleaved layout."""
    x = np.ascontiguousarray(np.transpose(np.asarray(in_pop_spikes, np.float32),
                                          (2, 1, 0)))  # [T, IN, B]
    B = x.shape[2]
    scale = (2.0 ** np.arange(T, dtype=np.float32)).reshape(T, 1, 1)
    xh32 = x.astype(np.float16).astype(np.float32)
    xa = (xh32 * scale).astype(np.float16)                 # exact 2^t * fp16(x)
    xr = ((x - xh32) * (scale * np.float32(2048.0))).astype(np.float16)

    com = {}
    for nm, W in (("w0p", W0), ("w1p", W1), ("wop", Wout)):
        WT = np.asarray(W, np.float32).T
        hi = WT.astype(np.float16)
        lo = ((WT - hi.astype(np.float32)) * np.float32(2048.0)).astype(np.float16)
        com[nm] = np.ascontiguousarray(
            np.concatenate([hi, lo], axis=1))            # [K, 2*H]
    b = np.concatenate([np.asarray(v, np.float32) for v in (b0, b1, bout)])
    com["brep2"] = np.ascontiguousarray(
        np.broadcast_to(np.float32(2.0) * b, (128, b.size)))
    bh = b.astype(np.float16)
    bl = ((b - bh.astype(np.float32)) * np.float32(2048.0)).astype(np.float16)
    com["negb"] = np.ascontiguousarray(np.stack([-bh, -bl]))
    o2 = np.zeros((2, 128), np.float16)
    o2[0] = 1.0
    o2[1] = np.float16(2.0 ** -11)
    com["ones2"] = o2

    in_maps = []
    for c in range(ncores):
        m = dict(com)
        xs = np.stack([xa[:, :, c * BL:(c + 1) * BL],
                       xr[:, :, c * BL:(c + 1) * BL]], axis=2)  # [T, IN, 2, BL]
        m["xp"] = np.ascontiguousarray(xs.reshape(T, xs.shape[1], 2 * BL))
        in_maps.append(m)
    return in_maps


def kernel(in_pop_spikes, W0, b0, W1, b1, Wout, bout, batch_size, _trace=False):
    T = in_pop_spikes.shape[2]
    nc = build(**FULL)
    in_maps = prep_inputs(in_pop_spikes, W0, b0, W1, b1, Wout, bout, T=T)
    res = run_bass_kernel_spmd(nc, in_maps, core_ids=list(range(NCORES)),
                               trace=_trace)
    out = np.concatenate([r["out"] for r in res.results], axis=0)
    out = (out / np.float32(T)).astype(np.float32)
    if _trace:
        kernel._last_results = res
    return out


# revision 3
# speedup vs baseline: 1.0116x; 1.0035x over previous
"""Trainium2 Bass kernel for nn_DynamicMLP (3-layer LIF spiking net, T=16).

Strategy (8 NeuronCores, data-parallel over batch):
  - Shard batch 1024 -> 8 x 128. Replicate weights. Zero cross-core comms.
  - Layout: [batch=128 partitions, hidden on free dim].
  - The LIF current c lives in PSUM scaled by 2^t:
      C_t = sum_{tau<=t} 2^tau * I_tau  ==  2^t * (c_t - bias part)
    Bias is factored out of the recursion (fixed point):
      c_t = 2^-t * C_t + (2 - 2^-t) * b
  - fp32-exact matmuls via fp16 hi/lo splits (fp16 x fp16 exact in fp32 PSUM):
      L0 (x inexact in fp16): 3 passes  xh@wh -> C0; xl@wh, xh@(wl*2^11) -> C0b
      L1/L2 (spikes exact):   2 passes  s@wh -> C;  (s*2^-11)@(wl*2^11) -> C
  - LIF algebra refactored to minimize the post-matmul critical chain:
      v_t = v0^2 - 0.172*U0 + c_t         (U == u / 0.172)
      U_t = 1.529*U0 - v0 ; U_{t+1} = U_t + (0.132/0.172)*s_t
    r = v0^2 - 0.172*U0 and r2 = r + (2-2^-t)*b precompute BEFORE C is ready,
    so the chain after the last matmul is just stt(C) -> spike -> transpose.
  - Elementwise work is spread over DVE / ACT / Pool engines; the PE runs only
    the 144 GEMM instructions per step (the precision-mandated minimum).
  - Host packs x and W hi/lo interleaved so every DMA row is a >=512B run.
"""
import sys

sys.path.insert(0, "/opt/trn_rl_repo")

import numpy as np

import concourse.bacc as bacc
import concourse.tile as tile
from concourse import mybir
from concourse.bass_utils import run_bass_kernel_spmd

dt = mybir.dt
F16 = dt.float16
F32 = dt.float32
Alu = mybir.AluOpType

NCORES = 8
FULL = dict(T=16, IN=2048, H0=1024, H1=1024, OUT=512, BL=128)
TH_V = -0.172
TH_U = 0.529
TH_S = 0.132
C_RESET = 0.021

_BUILD_CACHE = {}


def build(T=16, IN=2048, H0=1024, H1=1024, OUT=512, BL=128):
    key = (T, IN, H0, H1, OUT, BL)
    if key in _BUILD_CACHE:
        return _BUILD_CACHE[key]
    KT0, KT1, KT2 = IN // 128, H0 // 128, H1 // 128
    KH = KT0 // 2          # ktiles per x chunk
    NCH = 512              # psum bank free-dim (fp32)
    HS = {0: H0, 1: H1, 2: OUT}
    KTS = {1: KT1, 2: KT2}
    HTOT = H0 + H1 + OUT
    BOFF = {0: 0, 1: H0, 2: H0 + H1}

    nc = bacc.Bacc("TRN2", target_bir_lowering=False, debug=False, num_devices=NCORES)

    xp_d = nc.dram_tensor("xp", [T, IN, 2 * BL], F16, kind="ExternalInput")
    w_d = {"w0": nc.dram_tensor("w0p", [IN, 2 * H0], F16, kind="ExternalInput"),
           "w1": nc.dram_tensor("w1p", [H0, 2 * H1], F16, kind="ExternalInput"),
           "wo": nc.dram_tensor("wop", [H1, 2 * OUT], F16, kind="ExternalInput")}
    b_d = nc.dram_tensor("brep2", [128, HTOT], F32, kind="ExternalInput")
    nb_d = nc.dram_tensor("negb", [2, HTOT], F16, kind="ExternalInput")
    o2_d = nc.dram_tensor("ones2", [2, 128], F16, kind="ExternalInput")
    out_d = nc.dram_tensor("out", [BL, OUT], F32, kind="ExternalOutput")

    with tile.TileContext(nc) as tc:
        with tc.tile_pool(name="w", bufs=1) as wp, \
             tc.tile_pool(name="state", bufs=1) as sp, \
             tc.tile_pool(name="xs", bufs=2) as xp, \
             tc.tile_pool(name="spk", bufs=2) as kp, \
             tc.tile_pool(name="psum", bufs=1, space="PSUM") as pp:

            # ---- resident weight tiles (DMAs issued later, first-use order) --
            w_sb = {"w0": [wp.tile([128, 2 * H0], F16, tag=f"w0_{k}", name=f"w0_{k}")
                           for k in range(KT0)],
                    "w1": [wp.tile([128, 2 * H1], F16, tag=f"w1_{k}", name=f"w1_{k}")
                           for k in range(KT1)],
                    "wo": [wp.tile([128, 2 * OUT], F16, tag=f"wo_{k}", name=f"wo_{k}")
                           for k in range(KT2)]}

            def dma_w(nm, ks):
                for k in ks:
                    nc.sync.dma_start(out=w_sb[nm][k][:],
                                      in_=w_d[nm][k * 128:(k + 1) * 128, :])

            bfull = wp.tile([128, HTOT], F32, tag="bfull", name="bfull")  # 2*b
            negb = wp.tile([2, HTOT], F16, tag="negb", name="negb")
            ones2 = wp.tile([2, 128], F16, tag="ones2", name="ones2")

            # ---- states ----
            vm = {l: sp.tile([128, HS[l]], F32, tag=f"vm{l}", name=f"vm{l}")
                  for l in range(3)}
            vr = {l: sp.tile([128, HS[l]], F32, tag=f"vr{l}", name=f"vr{l}")
                  for l in range(3)}
            U = {l: sp.tile([128, HS[l]], F32, tag=f"U{l}", name=f"U{l}")
                 for l in range(3)}
            r2 = {l: sp.tile([128, HS[l]], F32, tag=f"r2{l}", name=f"r2{l}")
                  for l in range(3)}
            r = sp.tile([128, max(H0, H1)], F32, tag="r", name="r")
            c021 = sp.tile([128, max(H0, H1)], F32, tag="c021", name="c021")
            acc = sp.tile([128, OUT], F32, tag="acc", name="acc")

            C = {0: pp.tile([128, H0], F32, tag="C0", name="C0"),
                 1: pp.tile([128, H1], F32, tag="C1", name="C1"),
                 2: pp.tile([128, OUT], F32, tag="C2", name="C2")}
            C0b = pp.tile([128, H0], F32, tag="C0b", name="C0b")

            # ---- x tile loads (2 chunks per step; 512B dram runs) ----
            x_pre = {}

            def load_x(t):
                tiles = []
                for ci in range(2):
                    xt = xp.tile([128, KH * 2 * BL], F16, tag=f"x{ci}",
                                 name=f"x_t{t}_{ci}")
                    ks = ci * KH * 128
                    nc.sync.dma_start(
                        out=xt[:].rearrange("p (k b) -> p k b", b=2 * BL),
                        in_=xp_d[t:t + 1, ks:ks + KH * 128, :].rearrange(
                            "o (k p) b -> p (o k) b", p=128))
                    tiles.append(xt)
                x_pre[t] = tiles

            # ---- matmul emitters ----
            def emit_L0(t, ci):
                xt = x_pre[t][ci]
                if ci == 1:
                    x_pre.pop(t, None)
                for k in range(KH):
                    kg = ci * KH + k
                    la = xt[:, k * 256:k * 256 + 128]
                    lr = xt[:, k * 256 + 128:(k + 1) * 256]
                    wt = w_sb["w0"][kg]
                    for n0 in range(0, H0, NCH):
                        first = (t == 0 and kg == 0)
                        last = (t == T - 1 and kg == KT0 - 1)
                        ra = wt[:, n0:n0 + NCH]
                        rl = wt[:, H0 + n0:H0 + n0 + NCH]
                        nc.tensor.matmul(C[0][:, n0:n0 + NCH], la, ra, start=False,
                                         stop=last, skip_group_check=True)
                        psb = C0b[:, n0:n0 + NCH]
                        nc.tensor.matmul(psb, lr, ra, start=first, stop=False,
                                         skip_group_check=True)
                        nc.tensor.matmul(psb, la, rl, start=False, stop=last,
                                         skip_group_check=True)

            def emit_L(l, t, sT, sL):
                h = HS[l]
                for k in range(KTS[l]):
                    la = sT[:, k * 128:(k + 1) * 128]
                    lr = sL[:, k * 128:(k + 1) * 128]
                    wt = w_sb["w1" if l == 1 else "wo"][k]
                    for n0 in range(0, h, NCH):
                        nn = min(NCH, h - n0)
                        last = (t == T - 1 and k == KTS[l] - 1)
                        ps = C[l][:, n0:n0 + nn]
                        nc.tensor.matmul(ps, la, wt[:, n0:n0 + nn], start=False,
                                         stop=False, skip_group_check=True)
                        nc.tensor.matmul(ps, lr, wt[:, h + n0:h + n0 + nn],
                                         start=False, stop=last,
                                         skip_group_check=True)

            def emit_bias_init():
                """Seed C0/C1/C2 with -b via a K=2 matmul (rows 1, 2^-11).
                Then c_t = 2^-t*C_t + 2b for all t."""
                for l in range(3):
                    h = HS[l]
                    for n0 in range(0, h, NCH):
                        nn = min(NCH, h - n0)
                        nc.tensor.matmul(
                            C[l][:, n0:n0 + nn], ones2[:],
                            negb[:, BOFF[l] + n0:BOFF[l] + n0 + nn],
                            start=True, stop=False, skip_group_check=True)

            # ---- LIF pieces ----
            def emit_E(l, t):
                """Precompute r2_l = vr^2 - 0.172*U + 2b and U_t = 1.529*U - vr.
                Runs before C[l] is ready."""
                h = HS[l]
                nc.scalar.square(r[:, :h], vr[l][:])
                nc.vector.scalar_tensor_tensor(
                    out=r[:, :h], in0=U[l][:], scalar=TH_V, in1=r[:, :h],
                    op0=Alu.mult, op1=Alu.add)
                nc.gpsimd.tensor_tensor(
                    out=r2[l][:], in0=bfull[:, BOFF[l]:BOFF[l] + h],
                    in1=r[:, :h], op=Alu.add)
                nc.gpsimd.tensor_scalar(out=U[l][:], in0=U[l][:],
                                        scalar1=float(1.0 + TH_U), scalar2=None,
                                        op0=Alu.mult)
                nc.gpsimd.tensor_tensor(out=U[l][:], in0=U[l][:], in1=vr[l][:],
                                        op=Alu.subtract)

            def vhead(l, t):
                """v_t = 2^-t*C (+2^-(t+11)*C0b) + r2 — frees C[l] for t+1."""
                if l == 0:
                    nc.vector.scalar_tensor_tensor(
                        out=vm[0][:], in0=C0b[:], scalar=float(2.0 ** -(t + 11)),
                        in1=r2[0][:], op0=Alu.mult, op1=Alu.add)
                    nc.vector.scalar_tensor_tensor(
                        out=vm[0][:], in0=C[0][:], scalar=float(2.0 ** -t),
                        in1=vm[0][:], op0=Alu.mult, op1=Alu.add)
                else:
                    nc.vector.scalar_tensor_tensor(
                        out=vm[l][:], in0=C[l][:], scalar=float(2.0 ** -t),
                        in1=r2[l][:], op0=Alu.mult, op1=Alu.add)

            def vrest(l, t):
                """spike + state updates + E(l, t+1); returns (sT, sL) for l<2."""
                h = HS[l]
                last = (t == T - 1)
                if l == 2:
                    s2 = kp.tile([128, OUT], F32, tag="s2", name=f"s2_t{t}",
                                 bufs=1)
                    nc.vector.tensor_scalar(out=s2[:], in0=vm[2][:], scalar1=0.5,
                                            scalar2=1.0, op0=Alu.is_gt,
                                            op1=Alu.mult)
                    nc.gpsimd.tensor_tensor(out=acc[:], in0=acc[:], in1=s2[:],
                                            op=Alu.add)
                    if not last:
                        nc.vector.scalar_tensor_tensor(
                            out=U[2][:], in0=s2[:], scalar=float(TH_S / -TH_V),
                            in1=U[2][:], op0=Alu.mult, op1=Alu.add)
                        nc.scalar.copy(vr[2][:], vm[2][:])
                        nc.vector.copy_predicated(
                            out=vr[2][:], mask=s2[:].bitcast(dt.uint32),
                            data=c021[:, :h])
                        emit_E(2, t + 1)
                    return None, None
                sc = float(2.0 ** t)
                s = kp.tile([128, h], F16, tag="s", name=f"s{l}_t{t}")
                nc.vector.tensor_scalar(out=s[:], in0=vm[l][:], scalar1=0.5,
                                        scalar2=sc, op0=Alu.is_gt, op1=Alu.mult)
                sT = kp.tile([128, h], F16, tag="sT", name=f"sT{l}_t{t}")
                nc.scalar.dma_start_transpose(
                    out=sT[:].rearrange("p (k b) -> p k b", b=128), in_=s[:])
                sL = kp.tile([128, h], F16, tag="s", name=f"sL{l}_t{t}")
                nc.scalar.mul(sL[:], sT[:], float(2.0 ** -11))
                if not last:
                    nc.vector.scalar_tensor_tensor(
                        out=U[l][:], in0=s[:], scalar=float(TH_S / -TH_V / sc),
                        in1=U[l][:], op0=Alu.mult, op1=Alu.add)
                    nc.scalar.copy(vr[l][:], vm[l][:])
                    nc.vector.copy_predicated(
                        out=vr[l][:], mask=s[:].bitcast(dt.uint16),
                        data=c021[:, :h])
                    emit_E(l, t + 1)
                return sT, sL

            # ---- preamble: DMAs in first-use order + state init ----
            nc.sync.dma_start(out=ones2[:], in_=o2_d[:])
            nc.sync.dma_start(out=negb[:], in_=nb_d[:])
            emit_bias_init()
            load_x(0)
            dma_w("w0", range(0, KH))
            nc.vector.memset(c021[:], C_RESET)
            nc.gpsimd.memset(acc[:], 0.0)
            for l in range(3):
                nc.vector.memset(vr[l][:], 0.0)
                nc.gpsimd.memset(U[l][:], 0.0)
            dma_w("w0", range(KH, KT0))
            nc.sync.dma_start(out=bfull[:], in_=b_d[:])
            load_x(1)
            dma_w("w1", range(KT1))
            dma_w("wo", range(KT2))
            for l in range(3):
                emit_E(l, 0)

            # ---- steady state: 1-step layer skew ----
            def emit_rest(t):
                sT0, sL0 = vrest(0, t)
                emit_L(1, t, sT0, sL0)
                if t + 1 < T:
                    emit_L0(t + 1, 1)
                vhead(1, t)
                sT1, sL1 = vrest(1, t)
                emit_L(2, t, sT1, sL1)
                vhead(2, t)
                vrest(2, t)

            for t in range(T):
                if t >= 1:
                    vhead(0, t - 1)
                emit_L0(t, 0)
                if t >= 1:
                    emit_rest(t - 1)
                else:
                    emit_L0(0, 1)
                if t + 2 < T:
                    load_x(t + 2)
            # ---- final step: chunked spike chains + hi-then-lo GEMMs to cut
            # the PE's wait on s0T/s1T during the pipeline drain ----
            def final_chain(l, t):
                h = HS[l]
                sc = float(2.0 ** t)
                s = kp.tile([128, h], F16, tag="s", name=f"s{l}_t{t}")
                sT = kp.tile([128, h], F16, tag="sT", name=f"sT{l}_t{t}")
                for c in range(2):
                    sl = slice(c * NCH, (c + 1) * NCH)
                    if l == 0:
                        nc.vector.scalar_tensor_tensor(
                            out=vm[0][:, sl], in0=C0b[:, sl],
                            scalar=float(2.0 ** -(t + 11)), in1=r2[0][:, sl],
                            op0=Alu.mult, op1=Alu.add)
                        nc.vector.scalar_tensor_tensor(
                            out=vm[0][:, sl], in0=C[0][:, sl],
                            scalar=float(2.0 ** -t), in1=vm[0][:, sl],
                            op0=Alu.mult, op1=Alu.add)
                    else:
                        nc.vector.scalar_tensor_tensor(
                            out=vm[l][:, sl], in0=C[l][:, sl],
                            scalar=float(2.0 ** -t), in1=r2[l][:, sl],
                            op0=Alu.mult, op1=Alu.add)
                    nc.vector.tensor_scalar(out=s[:, sl], in0=vm[l][:, sl],
                                            scalar1=0.5, scalar2=sc,
                                            op0=Alu.is_gt, op1=Alu.mult)
                    kt = NCH // 128
                    nc.scalar.dma_start_transpose(
                        out=sT[:].rearrange("p (k b) -> p k b", b=128)
                            [:, c * kt:(c + 1) * kt, :],
                        in_=s[:, sl])
                sL = kp.tile([128, h], F16, tag="s", name=f"sL{l}_t{t}")
                nc.scalar.mul(sL[:], sT[:], float(2.0 ** -11))
                return sT, sL

            def emit_L_hilo(l, t, sT, sL):
                h = HS[l]
                wtag = {1: "w1", 2: "wo"}[l]
                for k in range(KTS[l]):
                    la = sT[:, k * 128:(k + 1) * 128]
                    wt = w_sb[wtag][k]
                    for n0 in range(0, h, NCH):
                        nn = min(NCH, h - n0)
                        nc.tensor.matmul(C[l][:, n0:n0 + nn], la,
                                         wt[:, n0:n0 + nn], start=False,
                                         stop=False, skip_group_check=True)
                for k in range(KTS[l]):
                    lr = sL[:, k * 128:(k + 1) * 128]
                    wt = w_sb[wtag][k]
                    for n0 in range(0, h, NCH):
                        nn = min(NCH, h - n0)
                        nc.tensor.matmul(C[l][:, n0:n0 + nn], lr,
                                         wt[:, h + n0:h + n0 + nn], start=False,
                                         stop=(t == T - 1 and k == KTS[l] - 1),
                                         skip_group_check=True)

            u = T - 1
            sT0, sL0 = final_chain(0, u)
            emit_L_hilo(1, u, sT0, sL0)
            sT1, sL1 = final_chain(1, u)
            emit_L_hilo(2, u, sT1, sL1)
            # final layer-2 chain straight on DVE, then output DMA
            sfin = kp.tile([128, OUT], F32, tag="s2", name="s2_final", bufs=1)
            nc.vector.scalar_tensor_tensor(
                out=vm[2][:], in0=C[2][:], scalar=float(2.0 ** -u),
                in1=r2[2][:], op0=Alu.mult, op1=Alu.add)
            nc.vector.tensor_scalar(out=sfin[:], in0=vm[2][:], scalar1=0.5,
                                    scalar2=1.0, op0=Alu.is_gt, op1=Alu.mult)
            nc.vector.scalar_tensor_tensor(
                out=acc[:], in0=sfin[:], scalar=1.0, in1=acc[:],
                op0=Alu.mult, op1=Alu.add)

            nc.sync.dma_start(out=out_d[:], in_=acc[:])

    nc.compile()
    _BUILD_CACHE[key] = nc
    return nc


def prep_inputs(in_pop_spikes, W0, b0, W1, b1, Wout, bout,
                T=16, BL=128, ncores=NCORES):
    """Host-side prep: pack hi/lo-split x and W with interleaved layout."""
    x = np.ascontiguousarray(np.transpose(np.asarray(in_pop_spikes, np.float32),
                                          (2, 1, 0)))  # [T, IN, B]
    B = x.shape[2]
    scale = (2.0 ** np.arange(T, dtype=np.float32)).reshape(T, 1, 1)
    xh32 = x.astype(np.float16).astype(np.float32)
    xa = (xh32 * scale).astype(np.float16)                 # exact 2^t * fp16(x)
    xr = ((x - xh32) * (scale * np.float32(2048.0))).astype(np.float16)

    com = {}
    for nm, W in (("w0p", W0), ("w1p", W1), ("wop", Wout)):
        WT = np.asarray(W, np.float32).T
        hi = WT.astype(np.float16)
        lo = ((WT - hi.astype(np.float32)) * np.float32(2048.0)).astype(np.float16)
        com[nm] = np.ascontiguousarray(
            np.concatenate([hi, lo], axis=1))            # [K, 2*H]
    b = np.concatenate([np.asarray(v, np.float32) for v in (b0, b1, bout)])
    com["brep2"] = np.ascontiguousarray(
        np.broadcast_to(np.float32(2.0) * b, (128, b.size)))
    bh = b.astype(np.float16)
    bl = ((b - bh.astype(np.float32)) * np.float32(2048.0)).astype(np.float16)
    com["negb"] = np.ascontiguousarray(np.stack([-bh, -bl]))
    o2 = np.zeros((2, 128), np.float16)
    o2[0] = 1.0
    o2[1] = np.float16(2.0 ** -11)
    com["ones2"] = o2

    in_maps = []
    for c in range(ncores):
        m = dict(com)
        xs = np.stack([xa[:, :, c * BL:(c + 1) * BL],
                       xr[:, :, c * BL:(c + 1) * BL]], axis=2)  # [T, IN, 2, BL]
        m["xp"] = np.ascontiguousarray(xs.reshape(T, xs.shape[1], 2 * BL))
        in_maps.append(m)
    return in_maps


def kernel(in_pop_spikes, W0, b0, W1, b1, Wout, bout, batch_size, _trace=False):
    T = in_pop_spikes.shape[2]
    nc = build(**FULL)
    in_maps = prep_inputs(in_pop_spikes, W0, b0, W1, b1, Wout, bout, T=T)
    res = run_bass_kernel_spmd(nc, in_maps, core_ids=list(range(NCORES)),
                               trace=_trace)
    out = np.concatenate([r["out"] for r in res.results], axis=0)
    out = (out / np.float32(T)).astype(np.float32)
    if _trace:
        kernel._last_results = res
    return out


# revision 4
# speedup vs baseline: 1.0174x; 1.0057x over previous
"""Trainium2 Bass kernel for nn_DynamicMLP (3-layer LIF spiking net, T=16).

Strategy (8 NeuronCores, data-parallel over batch):
  - Shard batch 1024 -> 8 x 128. Replicate weights. Zero cross-core comms.
  - Layout: [batch=128 partitions, hidden on free dim].
  - The LIF current c lives in PSUM scaled by 2^t:
      C_t = sum_{tau<=t} 2^tau * I_tau  ==  2^t * (c_t - bias part)
    Bias is factored out of the recursion (fixed point):
      c_t = 2^-t * C_t + (2 - 2^-t) * b
  - fp32-exact matmuls via fp16 hi/lo splits (fp16 x fp16 exact in fp32 PSUM):
      L0 (x inexact in fp16): 3 passes  xh@wh -> C0; xl@wh, xh@(wl*2^11) -> C0b
      L1/L2 (spikes exact):   2 passes  s@wh -> C;  (s*2^-11)@(wl*2^11) -> C
  - LIF algebra refactored to minimize the post-matmul critical chain:
      v_t = v0^2 - 0.172*U0 + c_t         (U == u / 0.172)
      U_t = 1.529*U0 - v0 ; U_{t+1} = U_t + (0.132/0.172)*s_t
    r = v0^2 - 0.172*U0 and r2 = r + (2-2^-t)*b precompute BEFORE C is ready,
    so the chain after the last matmul is just stt(C) -> spike -> transpose.
  - Elementwise work is spread over DVE / ACT / Pool engines; the PE runs only
    the 144 GEMM instructions per step (the precision-mandated minimum).
  - Host packs x and W hi/lo interleaved so every DMA row is a >=512B run.
"""
import sys

sys.path.insert(0, "/opt/trn_rl_repo")

import numpy as np

import concourse.bacc as bacc
import concourse.tile as tile
from concourse import mybir
from concourse.bass_utils import run_bass_kernel_spmd

dt = mybir.dt
F16 = dt.float16
F32 = dt.float32
Alu = mybir.AluOpType

NCORES = 8
FULL = dict(T=16, IN=2048, H0=1024, H1=1024, OUT=512, BL=128)
TH_V = -0.172
TH_U = 0.529
TH_S = 0.132
C_RESET = 0.021

_BUILD_CACHE = {}


def build(T=16, IN=2048, H0=1024, H1=1024, OUT=512, BL=128):
    key = (T, IN, H0, H1, OUT, BL)
    if key in _BUILD_CACHE:
        return _BUILD_CACHE[key]
    KT0, KT1, KT2 = IN // 128, H0 // 128, H1 // 128
    KH = KT0 // 2          # ktiles per x chunk
    NCH = 512              # psum bank free-dim (fp32)
    HS = {0: H0, 1: H1, 2: OUT}
    KTS = {1: KT1, 2: KT2}
    HTOT = H0 + H1 + OUT
    BOFF = {0: 0, 1: H0, 2: H0 + H1}

    nc = bacc.Bacc("TRN2", target_bir_lowering=False, debug=False, num_devices=NCORES)

    xp_d = nc.dram_tensor("xp", [T, IN, 2 * BL], F16, kind="ExternalInput")
    w_d = {"w0": nc.dram_tensor("w0p", [IN, 2 * H0], F16, kind="ExternalInput"),
           "w1": nc.dram_tensor("w1p", [H0, 2 * H1], F16, kind="ExternalInput"),
           "wo": nc.dram_tensor("wop", [H1, 2 * OUT], F16, kind="ExternalInput")}
    b_d = nc.dram_tensor("brep2", [128, HTOT], F32, kind="ExternalInput")
    nb_d = nc.dram_tensor("negb", [2, HTOT], F16, kind="ExternalInput")
    o2_d = nc.dram_tensor("ones2", [2, 128], F16, kind="ExternalInput")
    out_d = nc.dram_tensor("out", [BL, OUT], F32, kind="ExternalOutput")

    with tile.TileContext(nc) as tc:
        with tc.tile_pool(name="w", bufs=1) as wp, \
             tc.tile_pool(name="state", bufs=1) as sp, \
             tc.tile_pool(name="xs", bufs=2) as xp, \
             tc.tile_pool(name="spk", bufs=2) as kp, \
             tc.tile_pool(name="psum", bufs=1, space="PSUM") as pp:

            # ---- resident weight tiles (DMAs issued later, first-use order) --
            w_sb = {"w0": [wp.tile([128, 2 * H0], F16, tag=f"w0_{k}", name=f"w0_{k}")
                           for k in range(KT0)],
                    "w1": [wp.tile([128, 2 * H1], F16, tag=f"w1_{k}", name=f"w1_{k}")
                           for k in range(KT1)],
                    "wo": [wp.tile([128, 2 * OUT], F16, tag=f"wo_{k}", name=f"wo_{k}")
                           for k in range(KT2)]}

            def dma_w(nm, ks):
                for k in ks:
                    nc.sync.dma_start(out=w_sb[nm][k][:],
                                      in_=w_d[nm][k * 128:(k + 1) * 128, :])

            bfull = wp.tile([128, HTOT], F32, tag="bfull", name="bfull")  # 2*b
            negb = wp.tile([2, HTOT], F16, tag="negb", name="negb")
            ones2 = wp.tile([2, 128], F16, tag="ones2", name="ones2")

            # ---- states ----
            vm = {l: sp.tile([128, HS[l]], F32, tag=f"vm{l}", name=f"vm{l}")
                  for l in range(3)}
            vr = {l: sp.tile([128, HS[l]], F32, tag=f"vr{l}", name=f"vr{l}")
                  for l in range(3)}
            U = {l: sp.tile([128, HS[l]], F32, tag=f"U{l}", name=f"U{l}")
                 for l in range(3)}
            r2 = {l: sp.tile([128, HS[l]], F32, tag=f"r2{l}", name=f"r2{l}")
                  for l in range(3)}
            r = sp.tile([128, max(H0, H1)], F32, tag="r", name="r")
            c021 = sp.tile([128, max(H0, H1)], F32, tag="c021", name="c021")
            acc = sp.tile([128, OUT], F32, tag="acc", name="acc")

            C = {0: pp.tile([128, H0], F32, tag="C0", name="C0"),
                 1: pp.tile([128, H1], F32, tag="C1", name="C1"),
                 2: pp.tile([128, OUT], F32, tag="C2", name="C2")}
            C0b = pp.tile([128, H0], F32, tag="C0b", name="C0b")

            # ---- x tile loads (2 chunks per step; 512B dram runs) ----
            x_pre = {}

            def load_x(t):
                tiles = []
                for ci in range(2):
                    xt = xp.tile([128, KH * 2 * BL], F16, tag=f"x{ci}",
                                 name=f"x_t{t}_{ci}")
                    ks = ci * KH * 128
                    nc.sync.dma_start(
                        out=xt[:].rearrange("p (k b) -> p k b", b=2 * BL),
                        in_=xp_d[t:t + 1, ks:ks + KH * 128, :].rearrange(
                            "o (k p) b -> p (o k) b", p=128))
                    tiles.append(xt)
                x_pre[t] = tiles

            # ---- matmul emitters ----
            def emit_L0(t, ci):
                xt = x_pre[t][ci]
                if ci == 1:
                    x_pre.pop(t, None)
                for k in range(KH):
                    kg = ci * KH + k
                    la = xt[:, k * 256:k * 256 + 128]
                    lr = xt[:, k * 256 + 128:(k + 1) * 256]
                    wt = w_sb["w0"][kg]
                    for n0 in range(0, H0, NCH):
                        first = (t == 0 and kg == 0)
                        last = (t == T - 1 and kg == KT0 - 1)
                        ra = wt[:, n0:n0 + NCH]
                        rl = wt[:, H0 + n0:H0 + n0 + NCH]
                        nc.tensor.matmul(C[0][:, n0:n0 + NCH], la, ra, start=False,
                                         stop=last, skip_group_check=True)
                        psb = C0b[:, n0:n0 + NCH]
                        nc.tensor.matmul(psb, lr, ra, start=first, stop=False,
                                         skip_group_check=True)
                        nc.tensor.matmul(psb, la, rl, start=False, stop=last,
                                         skip_group_check=True)

            def emit_L(l, t, sT, sL):
                h = HS[l]
                for k in range(KTS[l]):
                    la = sT[:, k * 128:(k + 1) * 128]
                    lr = sL[:, k * 128:(k + 1) * 128]
                    wt = w_sb["w1" if l == 1 else "wo"][k]
                    for n0 in range(0, h, NCH):
                        nn = min(NCH, h - n0)
                        last = (t == T - 1 and k == KTS[l] - 1)
                        ps = C[l][:, n0:n0 + nn]
                        nc.tensor.matmul(ps, la, wt[:, n0:n0 + nn], start=False,
                                         stop=False, skip_group_check=True)
                        nc.tensor.matmul(ps, lr, wt[:, h + n0:h + n0 + nn],
                                         start=False, stop=last,
                                         skip_group_check=True)

            def emit_bias_init():
                """Seed C0/C1/C2 with -b via a K=2 matmul (rows 1, 2^-11).
                Then c_t = 2^-t*C_t + 2b for all t."""
                for l in range(3):
                    h = HS[l]
                    for n0 in range(0, h, NCH):
                        nn = min(NCH, h - n0)
                        nc.tensor.matmul(
                            C[l][:, n0:n0 + nn], ones2[:],
                            negb[:, BOFF[l] + n0:BOFF[l] + n0 + nn],
                            start=True, stop=False, skip_group_check=True)

            # ---- LIF pieces ----
            def emit_E(l, t):
                """Precompute r2_l = vr^2 - 0.172*U + 2b and U_t = 1.529*U - vr.
                Runs before C[l] is ready."""
                h = HS[l]
                nc.scalar.square(r[:, :h], vr[l][:])
                nc.vector.scalar_tensor_tensor(
                    out=r[:, :h], in0=U[l][:], scalar=TH_V, in1=r[:, :h],
                    op0=Alu.mult, op1=Alu.add)
                nc.gpsimd.tensor_tensor(
                    out=r2[l][:], in0=bfull[:, BOFF[l]:BOFF[l] + h],
                    in1=r[:, :h], op=Alu.add)
                nc.gpsimd.tensor_scalar(out=U[l][:], in0=U[l][:],
                                        scalar1=float(1.0 + TH_U), scalar2=None,
                                        op0=Alu.mult)
                nc.gpsimd.tensor_tensor(out=U[l][:], in0=U[l][:], in1=vr[l][:],
                                        op=Alu.subtract)

            def vhead(l, t):
                """v_t = 2^-t*C (+2^-(t+11)*C0b) + r2 — frees C[l] for t+1."""
                if l == 0:
                    nc.vector.scalar_tensor_tensor(
                        out=vm[0][:], in0=C0b[:], scalar=float(2.0 ** -(t + 11)),
                        in1=r2[0][:], op0=Alu.mult, op1=Alu.add)
                    nc.vector.scalar_tensor_tensor(
                        out=vm[0][:], in0=C[0][:], scalar=float(2.0 ** -t),
                        in1=vm[0][:], op0=Alu.mult, op1=Alu.add)
                else:
                    nc.vector.scalar_tensor_tensor(
                        out=vm[l][:], in0=C[l][:], scalar=float(2.0 ** -t),
                        in1=r2[l][:], op0=Alu.mult, op1=Alu.add)

            def vrest(l, t):
                """spike + state updates + E(l, t+1); returns (sT, sL) for l<2."""
                h = HS[l]
                last = (t == T - 1)
                if l == 2:
                    s2 = kp.tile([128, OUT], F32, tag="s2", name=f"s2_t{t}",
                                 bufs=1)
                    nc.vector.tensor_scalar(out=s2[:], in0=vm[2][:], scalar1=0.5,
                                            scalar2=1.0, op0=Alu.is_gt,
                                            op1=Alu.mult)
                    nc.gpsimd.tensor_tensor(out=acc[:], in0=acc[:], in1=s2[:],
                                            op=Alu.add)
                    if not last:
                        nc.vector.scalar_tensor_tensor(
                            out=U[2][:], in0=s2[:], scalar=float(TH_S / -TH_V),
                            in1=U[2][:], op0=Alu.mult, op1=Alu.add)
                        nc.scalar.copy(vr[2][:], vm[2][:])
                        nc.vector.copy_predicated(
                            out=vr[2][:], mask=s2[:].bitcast(dt.uint32),
                            data=c021[:, :h])
                        emit_E(2, t + 1)
                    return None, None
                sc = float(2.0 ** t)
                s = kp.tile([128, h], F16, tag="s", name=f"s{l}_t{t}")
                nc.vector.tensor_scalar(out=s[:], in0=vm[l][:], scalar1=0.5,
                                        scalar2=sc, op0=Alu.is_gt, op1=Alu.mult)
                sT = kp.tile([128, h], F16, tag="sT", name=f"sT{l}_t{t}")
                nc.scalar.dma_start_transpose(
                    out=sT[:].rearrange("p (k b) -> p k b", b=128), in_=s[:])
                sL = kp.tile([128, h], F16, tag="s", name=f"sL{l}_t{t}")
                nc.scalar.mul(sL[:], sT[:], float(2.0 ** -11))
                if not last:
                    nc.vector.scalar_tensor_tensor(
                        out=U[l][:], in0=s[:], scalar=float(TH_S / -TH_V / sc),
                        in1=U[l][:], op0=Alu.mult, op1=Alu.add)
                    nc.scalar.copy(vr[l][:], vm[l][:])
                    nc.vector.copy_predicated(
                        out=vr[l][:], mask=s[:].bitcast(dt.uint16),
                        data=c021[:, :h])
                    emit_E(l, t + 1)
                return sT, sL

            # ---- preamble: DMAs in first-use order + state init ----
            nc.sync.dma_start(out=ones2[:], in_=o2_d[:])
            nc.sync.dma_start(out=negb[:], in_=nb_d[:])
            emit_bias_init()
            load_x(0)
            dma_w("w0", range(0, KH))
            nc.vector.memset(c021[:], C_RESET)
            nc.gpsimd.memset(acc[:], 0.0)
            for l in range(3):
                nc.vector.memset(vr[l][:], 0.0)
                nc.gpsimd.memset(U[l][:], 0.0)
            dma_w("w0", range(KH, KT0))
            nc.sync.dma_start(out=bfull[:], in_=b_d[:])
            load_x(1)
            dma_w("w1", range(KT1))
            dma_w("wo", range(KT2))
            for l in range(3):
                emit_E(l, 0)

            # ---- steady state: 1-step layer skew ----
            def emit_rest(t):
                sT0, sL0 = vrest(0, t)
                emit_L(1, t, sT0, sL0)
                if t + 1 < T:
                    emit_L0(t + 1, 1)
                vhead(1, t)
                sT1, sL1 = vrest(1, t)
                emit_L(2, t, sT1, sL1)
                vhead(2, t)
                vrest(2, t)

            for t in range(T):
                if t >= 1:
                    vhead(0, t - 1)
                emit_L0(t, 0)
                if t >= 1:
                    emit_rest(t - 1)
                else:
                    emit_L0(0, 1)
                if t + 2 < T:
                    load_x(t + 2)
            # ---- final step: chunked spike chains + hi-then-lo GEMMs to cut
            # the PE's wait on s0T/s1T during the pipeline drain ----
            def final_chain(l, t):
                h = HS[l]
                sc = float(2.0 ** t)
                s = kp.tile([128, h], F16, tag="s", name=f"s{l}_t{t}")
                sT = kp.tile([128, h], F16, tag="sT", name=f"sT{l}_t{t}")
                FC = 256
                for c in range(h // FC):
                    sl = slice(c * FC, (c + 1) * FC)
                    if l == 0:
                        nc.vector.scalar_tensor_tensor(
                            out=vm[0][:, sl], in0=C0b[:, sl],
                            scalar=float(2.0 ** -(t + 11)), in1=r2[0][:, sl],
                            op0=Alu.mult, op1=Alu.add)
                        nc.vector.scalar_tensor_tensor(
                            out=vm[0][:, sl], in0=C[0][:, sl],
                            scalar=float(2.0 ** -t), in1=vm[0][:, sl],
                            op0=Alu.mult, op1=Alu.add)
                    else:
                        nc.vector.scalar_tensor_tensor(
                            out=vm[l][:, sl], in0=C[l][:, sl],
                            scalar=float(2.0 ** -t), in1=r2[l][:, sl],
                            op0=Alu.mult, op1=Alu.add)
                    nc.vector.tensor_scalar(out=s[:, sl], in0=vm[l][:, sl],
                                            scalar1=0.5, scalar2=sc,
                                            op0=Alu.is_gt, op1=Alu.mult)
                    kt = FC // 128
                    nc.scalar.dma_start_transpose(
                        out=sT[:].rearrange("p (k b) -> p k b", b=128)
                            [:, c * kt:(c + 1) * kt, :],
                        in_=s[:, sl])
                sL = kp.tile([128, h], F16, tag="s", name=f"sL{l}_t{t}")
                nc.scalar.mul(sL[:], sT[:], float(2.0 ** -11))
                return sT, sL

            def emit_L_hilo(l, t, sT, sL):
                h = HS[l]
                wtag = {1: "w1", 2: "wo"}[l]
                for k in range(KTS[l]):
                    la = sT[:, k * 128:(k + 1) * 128]
                    wt = w_sb[wtag][k]
                    for n0 in range(0, h, NCH):
                        nn = min(NCH, h - n0)
                        nc.tensor.matmul(C[l][:, n0:n0 + nn], la,
                                         wt[:, n0:n0 + nn], start=False,
                                         stop=False, skip_group_check=True)
                for k in range(KTS[l]):
                    lr = sL[:, k * 128:(k + 1) * 128]
                    wt = w_sb[wtag][k]
                    for n0 in range(0, h, NCH):
                        nn = min(NCH, h - n0)
                        nc.tensor.matmul(C[l][:, n0:n0 + nn], lr,
                                         wt[:, h + n0:h + n0 + nn], start=False,
                                         stop=(t == T - 1 and k == KTS[l] - 1),
                                         skip_group_check=True)

            u = T - 1
            sT0, sL0 = final_chain(0, u)
            emit_L_hilo(1, u, sT0, sL0)
            sT1, sL1 = final_chain(1, u)
            emit_L_hilo(2, u, sT1, sL1)
            # final layer-2 chain straight on DVE, then output DMA
            sfin = kp.tile([128, OUT], F32, tag="s2", name="s2_final", bufs=1)
            for c0 in range(0, OUT, 256):
                sl = slice(c0, c0 + 256)
                nc.vector.scalar_tensor_tensor(
                    out=vm[2][:, sl], in0=C[2][:, sl], scalar=float(2.0 ** -u),
                    in1=r2[2][:, sl], op0=Alu.mult, op1=Alu.add)
                nc.vector.tensor_scalar(out=sfin[:, sl], in0=vm[2][:, sl],
                                        scalar1=0.5, scalar2=1.0,
                                        op0=Alu.is_gt, op1=Alu.mult)
                nc.vector.scalar_tensor_tensor(
                    out=acc[:, sl], in0=sfin[:, sl], scalar=1.0,
                    in1=acc[:, sl], op0=Alu.mult, op1=Alu.add)
                nc.sync.dma_start(out=out_d[:, sl], in_=acc[:, sl])

    nc.compile()
    _BUILD_CACHE[key] = nc
    return nc


def prep_inputs(in_pop_spikes, W0, b0, W1, b1, Wout, bout,
                T=16, BL=128, ncores=NCORES):
    """Host-side prep: pack hi/lo-split x and W with interleaved layout."""
    x = np.ascontiguousarray(np.transpose(np.asarray(in_pop_spikes, np.float32),
                                          (2, 1, 0)))  # [T, IN, B]
    B = x.shape[2]
    scale = (2.0 ** np.arange(T, dtype=np.float32)).reshape(T, 1, 1)
    xh32 = x.astype(np.float16).astype(np.float32)
    xa = (xh32 * scale).astype(np.float16)                 # exact 2^t * fp16(x)
    xr = ((x - xh32) * (scale * np.float32(2048.0))).astype(np.float16)

    com = {}
    for nm, W in (("w0p", W0), ("w1p", W1), ("wop", Wout)):
        WT = np.asarray(W, np.float32).T
        hi = WT.astype(np.float16)
        lo = ((WT - hi.astype(np.float32)) * np.float32(2048.0)).astype(np.float16)
        com[nm] = np.ascontiguousarray(
            np.concatenate([hi, lo], axis=1))            # [K, 2*H]
    b = np.concatenate([np.asarray(v, np.float32) for v in (b0, b1, bout)])
    com["brep2"] = np.ascontiguousarray(
        np.broadcast_to(np.float32(2.0) * b, (128, b.size)))
    bh = b.astype(np.float16)
    bl = ((b - bh.astype(np.float32)) * np.float32(2048.0)).astype(np.float16)
    com["negb"] = np.ascontiguousarray(np.stack([-bh, -bl]))
    o2 = np.zeros((2, 128), np.float16)
    o2[0] = 1.0
    o2[1] = np.float16(2.0 ** -11)
    com["ones2"] = o2

    in_maps = []
    for c in range(ncores):
        m = dict(com)
        xs = np.stack([xa[:, :, c * BL:(c + 1) * BL],
                       xr[:, :, c * BL:(c + 1) * BL]], axis=2)  # [T, IN, 2, BL]
        m["xp"] = np.ascontiguousarray(xs.reshape(T, xs.shape[1], 2 * BL))
        in_maps.append(m)
    return in_maps


def kernel(in_pop_spikes, W0, b0, W1, b1, Wout, bout, batch_size, _trace=False):
    T = in_pop_spikes.shape[2]
    nc = build(**FULL)
    in_maps = prep_inputs(in_pop_spikes, W0, b0, W1, b1, Wout, bout, T=T)
    res = run_bass_kernel_spmd(nc, in_maps, core_ids=list(range(NCORES)),
                               trace=_trace)
    out = np.concatenate([r["out"] for r in res.results], axis=0)
    out = (out / np.float32(T)).astype(np.float32)
    if _trace:
        kernel._last_results = res
    return out
